# revision 28
# baseline (speedup 1.0000x reference)
"""DeeperGCN (GENConv softmax-aggr, L=2) Trainium2 kernel, 8-core SPMD.

Strategy (v2):
  - Nodes 1D-partitioned: core k owns 6250 nodes (padded to 6272 = 49*128).
  - Per layer, each core computes node-level message tables
    ew = [exp(m) | m*exp(m)] with m = relu(h)+eps for its shard (fp16),
    AllGathers the full [50176, 256] table, then processes its incident
    edges (grouped by dst block of 128 nodes) with:
      indirect-DMA row gather  ->  one-hot matmul scatter-accumulate in PSUM.
    softmax aggregate = wsum/ssum computed as exp(ln(wsum)-ln(ssum)).
  - The one-hot scatter matrices are PRECOMPUTED ON HOST (pure graph data)
    and streamed in via cheap static DMA; this frees gpsimd/vector, whose
    is_equal one-hot builds dominated v1.
  - Gather descriptors: per-(group,segment) exact valid counts (trailing
    -1 indices are skipped by the SWDGE microcode), queue cycling 0..3.
  - MLP single-pass: h1 = X@w1 stored fp16; BN stats from the fp32 PSUM via
    bn_stats (count-weighted combine, pads are zero) + AllReduce of raw
    sums; pass2 reads h1. All MLP matmuls fp16 (4x PE throughput vs fp32).
  - LayerNorm per node after PE transpose back to node-major (fp16 data,
    fp32 accumulators).
"""

import os
import sys
import math

import numpy as np

sys.path.insert(0, "/opt/trn_rl_repo")

# Problem constants (hardcoded per contract)
N = 50000
E_EDGES = 640000
D = 128
D2 = 256
L = 2
C_IN = 128
C_OUT = 64
MSG_EPS = 1e-7
W = 8           # cores
P = 128         # partitions
SH_REAL = N // W          # 6250 real nodes per core
NB = math.ceil(SH_REAL / P)   # 49 node blocks per core
SH = NB * P               # 6272 padded nodes per core
NPAD = SH * W             # 50176
MT = 512                  # MLP node-tile width


def default_params():
    return dict(
        W=W, P=P, D=D, D2=D2, L=L, C_OUT=C_OUT, SH=SH, SH_REAL=SH_REAL,
        NB=NB, NPAD=NPAD, MT=MT, MSG_EPS=MSG_EPS,
        CAPL=8, CAPH=8, GRP=2, HB=24,
        VL=(), VH=(),
        exact_counts=os.environ.get("K_EXACT", "1") == "1",
        single_packet=os.environ.get("K_SP", "0") == "1",
        queue_cycle=os.environ.get("K_QC", "1") == "1",
        # fast-path flags (host-verified against actual input values)
        t_one=True, in_b_zero=True, out_b_zero=True, ln_identity=True,
        b2_zero=True,
    )


def build_program(p):
    from concourse import bacc, bass, mybir, tile
    from concourse.masks import make_identity
    from contextlib import ExitStack

    dt = mybir.dt
    f32, f16, i32 = dt.float32, dt.float16, dt.int32
    i16 = dt.int16
    AF = mybir.ActivationFunctionType
    OP = mybir.AluOpType

    Wn, Pn, Dn, D2n = p["W"], p["P"], p["D"], p["D2"]
    Ln, COUT = p["L"], p["C_OUT"]
    SHn, SHR, NBn, NPADn = p["SH"], p["SH_REAL"], p["NB"], p["NPAD"]
    MTn = p["MT"]
    CAPA, CAPB, GRP, HB = p["CAPL"], p["CAPH"], p["GRP"], p["HB"]
    CAP = CAPA + CAPB
    AROWS = HB * Pn               # half-A table rows per core (3072)
    BROWS = (NBn - HB) * Pn       # half-B rows per core (3200)
    NG = math.ceil(NBn / GRP)     # gather groups
    NMT = math.ceil(SHn / MTn)    # mlp node tiles
    VA, VB = p["VL"], p["VH"]     # per-block exact valid gather counts
    PF = 2                        # A-gather prefetch depth (groups)
    eps_msg = p["MSG_EPS"]
    PADP0 = SHR - (NBn - 1) * Pn  # 106

    nc = bacc.Bacc(
        "TRN2", target_bir_lowering=False, debug=False,
        enable_asserts=False, num_devices=Wn, num_swdge_queues=4,
    )

    def din(name, shape, dty):
        return nc.dram_tensor(name, shape, dty, kind="ExternalInput").ap()

    x_fm_d = din("x_fm", [Dn, SHn], f16)            # host-transposed x shard
    idx16_d = din("idx16", [NG, Pn, GRP * CAP * 8], i16)  # gather indices
    # per-block layout within a group row: [b0: CAP*8 | b1: CAP*8],
    # each block's units: [lo: CAPL*8 | hi: CAPH*8]
    soh_d = din("s_onehot", [NG, Pn, GRP * CAP * Pn], f16)  # one-hot scatter
    in_w_d = din("in_w", [Dn, Dn], f16)
    w1_d = din("w1", [Ln, Dn, D2n], f16)
    w2_d = din("w2", [Ln, D2n, Dn], f16)
    bn_g_d = din("bn_g", [Ln, D2n], f32)
    bn_b_d = din("bn_b", [Ln, D2n], f32)
    out_w_d = din("out_w", [Dn, COUT], f32)
    if not p["b2_zero"]:
        b2_d = din("b2", [Ln, Dn], f32)
    if not p["t_one"]:
        t_d = din("t", [Ln], f32)
    if not p["in_b_zero"]:
        in_b_d = din("in_b", [Dn], f32)
    if not p["out_b_zero"]:
        out_b_d = din("out_b", [COUT], f32)
    if not p["ln_identity"]:
        ln_g_d = din("ln_g", [Ln, Dn], f32)
        ln_b_d = din("ln_b", [Ln, Dn], f32)

    out_d = nc.dram_tensor("out", [SHn, COUT], f32, kind="ExternalOutput").ap()

    rg = [list(range(Wn))]

    with ExitStack() as ctx:
        tc = ctx.enter_context(tile.TileContext(nc))
        sb = ctx.enter_context(tc.tile_pool(name="sb", bufs=1))
        sb2 = ctx.enter_context(tc.tile_pool(name="sb2", bufs=2))
        sb3 = ctx.enter_context(tc.tile_pool(name="sb3", bufs=3))
        pp = ctx.enter_context(tc.tile_pool(name="pp", bufs=2, space="PSUM"))
        dr = ctx.enter_context(tc.tile_pool(name="dr", bufs=2, space="DRAM"))

        # ---- constants / weights resident in SBUF ----
        ident = sb.tile([Pn, Pn], f32, tag="ident")
        make_identity(nc, ident[:])
        ident16 = sb.tile([Pn, Pn], f16, tag="ident16")
        nc.vector.tensor_copy(ident16[:], ident[:])

        in_w_sb = sb.tile([Pn, Dn], f16, tag="in_w")
        nc.sync.dma_start(out=in_w_sb[:], in_=in_w_d)
        w1_sb = sb.tile([Pn, Ln, D2n], f16, tag="w1")
        w2_sb = sb.tile([Pn, Ln, 2, Dn], f16, tag="w2")
        bng_sb = sb.tile([Pn, Ln, 2], f32, tag="bng")
        bnb_sb = sb.tile([Pn, Ln, 2], f32, tag="bnb")
        for l in range(Ln):
            nc.sync.dma_start(out=w1_sb[:, l, :], in_=w1_d[l])
            for ch in range(2):
                nc.sync.dma_start(out=w2_sb[:, l, ch, :],
                                  in_=w2_d[l, ch * Pn:(ch + 1) * Pn, :])
            nc.sync.dma_start(
                out=bng_sb[:, l, :],
                in_=bn_g_d[l].rearrange("(c p) -> p c", p=Pn))
            nc.sync.dma_start(
                out=bnb_sb[:, l, :],
                in_=bn_b_d[l].rearrange("(c p) -> p c", p=Pn))
        out_w_sb = sb.tile([Pn, COUT], f32, tag="out_w")
        nc.sync.dma_start(out=out_w_sb[:], in_=out_w_d)

        ones_row = sb.tile([1, Pn], f32, tag="ones_row")
        nc.vector.memset(ones_row[:], 1.0)

        def const_col(val, tagname):
            tcol = sb.tile([Pn, 1], f32, tag=tagname)
            nc.vector.memset(tcol[:], val)
            return tcol

        c_1e16 = const_col(1e-16, "c_1e16")
        c_1e30 = const_col(1e-30, "c_1e30")
        c_1e5 = const_col(1e-5, "c_1e5")

        # per-partition mask: 1.0 for real nodes of the last block, 0.0 for
        # the 22 pad nodes (partition offsets are illegal for engine ops, so
        # pad zeroing is a masked multiply of the whole block)
        iota_p = sb.tile([Pn, 1], i32, tag="iota_p")
        nc.gpsimd.iota(iota_p[:], pattern=[[1, 1]], base=0,
                       channel_multiplier=1)
        padmask = sb.tile([Pn, 1], f32, tag="padmask")
        nc.vector.tensor_scalar(out=padmask[:], in0=iota_p[:],
                                scalar1=float(PADP0) - 0.5, scalar2=None,
                                op0=OP.is_lt)

        def zero_h_pads():
            nc.vector.tensor_scalar(
                out=h_sb[:, NBn - 1, :], in0=h_sb[:, NBn - 1, :],
                scalar1=padmask[:, 0:1], scalar2=None, op0=OP.mult)

        def bcast_row(dram_row_ap, width, tagname):
            """[1,width] dram -> [128,width] sbuf via ones-matmul."""
            row = sb.tile([1, width], f32, tag=tagname + "_r")
            nc.sync.dma_start(out=row[:], in_=dram_row_ap)
            ps = pp.tile([Pn, width], f32, tag="psm", name=tagname + "_ps")
            nc.tensor.matmul(ps[:], lhsT=ones_row[:], rhs=row[:],
                             start=True, stop=True)
            out = sb.tile([Pn, width], f32, tag=tagname)
            nc.scalar.activation(out[:], ps[:], AF.Copy)
            return out

        if not p["b2_zero"]:
            b2c_sb = sb.tile([Pn, Ln], f32, tag="b2c")
            for l in range(Ln):
                nc.sync.dma_start(out=b2c_sb[:, l:l + 1], in_=b2_d[l][:, None])
        if not p["t_one"]:
            t_bc = bcast_row(t_d[None, :], Ln, "t_bc")  # [128, L]
        if not p["in_b_zero"]:
            inb_bc = bcast_row(in_b_d[None, :], Dn, "inb_bc")
        if not p["out_b_zero"]:
            outb_bc = bcast_row(out_b_d[None, :], COUT, "outb_bc")
        if not p["ln_identity"]:
            lng_bc = [bcast_row(ln_g_d[l][None, :], Dn, f"lng{l}")
                      for l in range(Ln)]
            lnb_bc = [bcast_row(ln_b_d[l][None, :], Dn, f"lnb{l}")
                      for l in range(Ln)]

        qcounter = [0]

        def next_q():
            q = qcounter[0] % 4 if p["queue_cycle"] else 0
            qcounter[0] += 1
            return q

        # ---- persistent state ----
        h_sb = sb.tile([Pn, NBn, Dn], f32, tag="h")      # node-major h shard
        X_fm = sb.tile([Pn, SHn], f16, tag="Xfm")        # feature-major agg+h
        h1_sb = sb.tile([Pn, 2, SHn], f16, tag="h1")     # mlp hidden (fp16)
        h2T_sb = sb.tile([Pn, NBn, Dn], f16, tag="h2T")  # node-major h2
        out_sb = sb.tile([Pn, NBn, COUT], f32, tag="out_sb")

        # Pre-zero all GW buffers (stale fp16 bits could encode inf/nan,
        # and pad one-hot rows rely on finite garbage * 0 == 0).
        for _ in range(3):
            gw0 = sb3.tile([Pn, GRP * CAP, 2 * Dn], f16, tag="gw", name="GW")
            nc.vector.memset(gw0[:], 0.0)

        # message-table build for a 4-block tile: ew = [exp(m) | m*exp(m)]
        ew_sb = sb.tile([Pn, NBn, 2, Dn], f16, tag="ew", name="ew_sb")

        def build_tables(t4, w4, l):
            m_sb = sb2.tile([Pn, 4, Dn], f32, tag="m", name="m_sb")
            nc.vector.tensor_scalar(
                out=m_sb[:, :w4, :], in0=h_sb[:, t4:t4 + w4, :],
                scalar1=0.0, scalar2=eps_msg, op0=OP.max, op1=OP.add)
            if p["t_one"]:
                nc.scalar.activation(ew_sb[:, t4:t4 + w4, 0, :],
                                     m_sb[:, :w4, :], AF.Exp)
            else:
                nc.scalar.activation(ew_sb[:, t4:t4 + w4, 0, :],
                                     m_sb[:, :w4, :], AF.Exp,
                                     scale=t_bc[:, l:l + 1])
            nc.vector.tensor_mul(ew_sb[:, t4:t4 + w4, 1, :],
                                 m_sb[:, :w4, :],
                                 ew_sb[:, t4:t4 + w4, 0, :])

        # emit the half-table shard DMA + AllGather; returns the full tile
        def table_allgather(half):
            if half == 0:
                rows, bsl = AROWS, ew_sb[:, 0:HB, :, :]
            else:
                rows, bsl = BROWS, ew_sb[:, HB:NBn, :, :]
            shard = dr.tile([rows, 2 * Dn], f16, tag=f"ew_shard{half}",
                            name="ew_shard")
            nc.sync.dma_start(
                out=shard[:].rearrange("(b p) f -> p b f", p=Pn),
                in_=bsl)
            full = dr.tile([rows * Wn, 2 * Dn], f16, tag=f"ew_full{half}",
                           addr_space="Shared", name="ew_full")
            nc.gpsimd.collective_compute(
                "AllGather", OP.bypass, replica_groups=rg,
                ins=[shard[:]], outs=[full[:]])
            return full

        # ---- in-projection: h0 = x @ in_w (+ in_b) ----
        # X_fm doubles as the staging buffer for the transposed x shard;
        # the layer-0 edge phase overwrites it only after in-proj reads it.
        nc.sync.dma_start(out=X_fm[:], in_=x_fm_d)
        for b in range(NBn):
            h0_ps = pp.tile([Pn, Dn], f32, tag="psm", name="h0_ps")
            nc.tensor.matmul(h0_ps[:], lhsT=X_fm[:, b * Pn:(b + 1) * Pn],
                             rhs=in_w_sb[:], start=True, stop=True)
            nc.scalar.activation(h_sb[:, b, :], h0_ps[:], AF.Copy)
            if not p["in_b_zero"]:
                nc.vector.tensor_add(h_sb[:, b, :], h_sb[:, b, :], inb_bc[:])
        if not p["in_b_zero"]:
            zero_h_pads()
        # layer-0 tables; AllGather A fires as soon as its blocks are built
        for t4 in range(0, HB, 4):
            build_tables(t4, 4, 0)
        ewA_full = table_allgather(0)
        for t4 in range(HB, NBn, 4):
            build_tables(t4, min(4, NBn - t4), 0)
        ewB_full = table_allgather(1)

        # ---- layers ----
        for l in range(Ln):
            # -- edge aggregation: per group of GRP dst blocks, one
            #    dma_gather per (block, table-half), then per block one-hot
            #    matmul accumulation. A-gathers run PF groups ahead of the
            #    B-gathers so the edge phase starts while AllGather-B is
            #    still in flight. MLP pass-1 matmuls + BN stats interleave
            #    as node tiles complete. --
            def mlp_pass1(i):
                w_i = min(MTn, SHn - i * MTn)
                xs = X_fm[:, i * MTn:i * MTn + w_i]
                for ch in range(2):
                    p1 = pp.tile([Pn, MTn], f32, tag="mm1", name="p1s")
                    nc.tensor.matmul(
                        p1[:, :w_i],
                        lhsT=w1_sb[:, l, ch * Pn:(ch + 1) * Pn],
                        rhs=xs, start=True, stop=True)
                    nc.scalar.activation(
                        h1_sb[:, ch, i * MTn:i * MTn + w_i],
                        p1[:, :w_i], AF.Copy)
                    nc.vector.bn_stats(stats6[:, ch, i, :], p1[:, :w_i])

            stats6 = sb.tile([Pn, 2, NMT, 6], f32, tag="stats6",
                             name="stats6")
            gstate = {}

            def emit_pre(g):
                nblk = min(GRP, NBn - g * GRP)
                idxt = sb3.tile([Pn, GRP * CAP * 8], i16, tag="idxt",
                                name="idxt")
                nc.sync.dma_start(out=idxt[:], in_=idx16_d[g])
                Sg = sb3.tile([Pn, GRP * CAP, Pn], f16, tag="sg", name="Sg")
                nc.sync.dma_start(
                    out=Sg[:],
                    in_=soh_d[g].rearrange("p (c j) -> p c j", j=Pn))
                GW = sb3.tile([Pn, GRP * CAP, 2 * Dn], f16, tag="gw",
                              name="GW")
                gstate[g] = (idxt, Sg, GW)
                for s_ in range(nblk):
                    b = g * GRP + s_
                    u0 = s_ * CAP * 8
                    nc.gpsimd.dma_gather(
                        out_ap=GW[:, s_ * CAPA:(s_ + 1) * CAPA, :],
                        in_ap=ewA_full[:],
                        idxs_ap=idxt[:, u0:u0 + CAPA * 8],
                        num_idxs=CAPA * Pn, num_idxs_reg=VA[b],
                        elem_size=2 * Dn,
                        single_packet=p["single_packet"], queue_num=next_q())

            def emit_post(g):
                nblk = min(GRP, NBn - g * GRP)
                idxt, Sg, GW = gstate.pop(g)
                for s_ in range(nblk):
                    b = g * GRP + s_
                    u0 = s_ * CAP * 8
                    nc.gpsimd.dma_gather(
                        out_ap=GW[:, GRP * CAPA + s_ * CAPB:
                                  GRP * CAPA + (s_ + 1) * CAPB, :],
                        in_ap=ewB_full[:],
                        idxs_ap=idxt[:, u0 + CAPA * 8:u0 + CAP * 8],
                        num_idxs=CAPB * Pn, num_idxs_reg=VB[b],
                        elem_size=2 * Dn,
                        single_packet=p["single_packet"],
                        queue_num=next_q())
                for s_ in range(nblk):
                    b = g * GRP + s_
                    pblk = pp.tile([Pn, 2 * Dn], f32, tag="pblk", name="pblk")
                    for c in range(CAP):
                        gc = (s_ * CAPA + c) if c < CAPA else (
                            GRP * CAPA + s_ * CAPB + (c - CAPA))
                        nc.tensor.matmul(pblk[:], lhsT=Sg[:, gc, :],
                                         rhs=GW[:, gc, :],
                                         start=(c == 0), stop=(c == CAP - 1))
                    # agg = wsum/(ssum+1e-16) = exp(ln(wsum) - ln(ssum+eps))
                    ln_e = sb2.tile([Pn, Dn], f32, tag="lne", name="ln_e")
                    nc.scalar.activation(ln_e[:], pblk[:, 0:Dn], AF.Ln,
                                         bias=c_1e16[:])
                    ln_w = sb2.tile([Pn, Dn], f32, tag="lnw", name="ln_w")
                    nc.scalar.activation(ln_w[:], pblk[:, Dn:2 * Dn], AF.Ln,
                                         bias=c_1e30[:])
                    dlog = sb2.tile([Pn, Dn], f32, tag="dlog", name="dlog")
                    nc.vector.tensor_sub(dlog[:], ln_w[:], ln_e[:])
                    Xnm = sb2.tile([Pn, Dn], f32, tag="Xnm", name="Xnm")
                    nc.scalar.activation(Xnm[:], dlog[:], AF.Exp)
                    nc.vector.tensor_add(Xnm[:], Xnm[:], h_sb[:, b, :])
                    xT_ps = pp.tile([Pn, Dn], f32, tag="psm", name="xT_ps")
                    nc.tensor.transpose(xT_ps[:], Xnm[:], ident[:])
                    nc.scalar.activation(X_fm[:, b * Pn:(b + 1) * Pn],
                                         xT_ps[:], AF.Copy)

            for gg in range(NG + PF):
                if gg < NG:
                    emit_pre(gg)
                if gg >= PF:
                    g = gg - PF
                    emit_post(g)
                    # node tile i spans blocks 4i..4i+3 = groups 2i, 2i+1
                    if g % 2 == 1:
                        mlp_pass1(g // 2)
            mlp_pass1(NMT - 1)   # last tile (block 48, group 24)

            # -- BN stats combine + AllReduce of raw sums --
            mv = sb2.tile([Pn, 2, 2], f32, tag="mv", name="mv")
            for ch in range(2):
                nc.vector.bn_aggr(mv[:, ch, :], stats6[:, ch, :, :])
            # pack raw sums [S1_0, S1_1, S2_0, S2_1]; S1 = n*mean,
            # S2 = n*(var + mean^2), n = 6272 (pads contribute ~0)
            bnar_sb = sb2.tile([Pn, 4], f32, tag="bnar", name="bnar_sb")
            nc.vector.tensor_scalar(out=bnar_sb[:, 0:2], in0=mv[:, :, 0],
                                    scalar1=float(SHn), scalar2=None,
                                    op0=OP.mult)
            m2t = sb2.tile([Pn, 2], f32, tag="m2t", name="m2t")
            nc.vector.tensor_mul(m2t[:], mv[:, :, 0], mv[:, :, 0])
            nc.vector.tensor_add(m2t[:], mv[:, :, 1], m2t[:])
            nc.vector.tensor_scalar(out=bnar_sb[:, 2:4], in0=m2t[:],
                                    scalar1=float(SHn), scalar2=None,
                                    op0=OP.mult)
            bnar_in = dr.tile([Pn, 4], f32, tag="bnar_in", name="bnar_in")
            nc.sync.dma_start(out=bnar_in[:], in_=bnar_sb[:])
            bnar_out = dr.tile([Pn, 4], f32, tag="bnar_out",
                               addr_space="Shared", name="bnar_out")
            nc.gpsimd.collective_compute(
                "AllReduce", OP.add, replica_groups=rg,
                ins=[bnar_in[:]], outs=[bnar_out[:]])
            gsb = sb2.tile([Pn, 4], f32, tag="gsb", name="gsb")
            nc.sync.dma_start(out=gsb[:], in_=bnar_out[:])
            mg = sb2.tile([Pn, 2], f32, tag="mg", name="mg")
            nc.vector.tensor_scalar(out=mg[:], in0=gsb[:, 0:2],
                                    scalar1=1.0 / N, scalar2=None,
                                    op0=OP.mult)
            ex2 = sb2.tile([Pn, 2], f32, tag="ex2", name="ex2")
            nc.vector.tensor_scalar(out=ex2[:], in0=gsb[:, 2:4],
                                    scalar1=1.0 / N, scalar2=None,
                                    op0=OP.mult)
            varb = sb2.tile([Pn, 2], f32, tag="varb", name="varb")
            nc.vector.tensor_mul(varb[:], mg[:], mg[:])
            nc.vector.tensor_sub(varb[:], ex2[:], varb[:])
            lv = sb2.tile([Pn, 2], f32, tag="lv", name="lv")
            nc.scalar.activation(lv[:], varb[:], AF.Ln, bias=c_1e5[:])
            rstd = sb2.tile([Pn, 2], f32, tag="rstd", name="rstd")
            nc.scalar.activation(rstd[:], lv[:], AF.Exp, scale=-0.5)
            sc_a = sb2.tile([Pn, 2], f32, tag="sc_a", name="sc_a")
            nc.vector.tensor_mul(sc_a[:], bng_sb[:, l, :], rstd[:])
            bi_a = sb2.tile([Pn, 2], f32, tag="bi_a", name="bi_a")
            nc.vector.tensor_mul(bi_a[:], mg[:], sc_a[:])
            nc.vector.tensor_sub(bi_a[:], bnb_sb[:, l, :], bi_a[:])

            # -- MLP pass 2 (from stored h1) + per-tile LayerNorm, residual,
            #    and either next-layer table build (+ split AllGathers) or
            #    the output projection --
            ln_sum = sb.tile([Pn, NBn], f32, tag="ln_sum", name="ln_sum")
            ln_sq = sb.tile([Pn, NBn], f32, tag="ln_sq", name="ln_sq")
            last = l == Ln - 1
            for i in range(NMT):
                w_i = min(MTn, SHn - i * MTn)
                nbt = w_i // Pn
                b0 = (i * MTn) // Pn
                sl = slice(b0, b0 + nbt)
                hbn = []
                for ch in range(2):
                    hb = sb2.tile([Pn, MTn], f16, tag=f"hbn{ch}",
                                  name="hb")
                    nc.scalar.activation(
                        hb[:, :w_i], h1_sb[:, ch, i * MTn:i * MTn + w_i],
                        AF.Relu, scale=sc_a[:, ch:ch + 1],
                        bias=bi_a[:, ch:ch + 1])
                    hbn.append(hb)
                p2 = pp.tile([Pn, MTn], f32, tag="mm2", name="p2")
                for ch in range(2):
                    nc.tensor.matmul(p2[:, :w_i], lhsT=w2_sb[:, l, ch, :],
                                     rhs=hbn[ch][:, :w_i],
                                     start=(ch == 0), stop=(ch == 1))
                h2c = sb2.tile([Pn, MTn], f16, tag="h2c", name="h2c")
                if p["b2_zero"]:
                    nc.scalar.activation(h2c[:, :w_i], p2[:, :w_i], AF.Copy)
                else:
                    nc.scalar.activation(h2c[:, :w_i], p2[:, :w_i],
                                         AF.Identity,
                                         bias=b2c_sb[:, l:l + 1])
                for j in range(nbt):
                    st = b0 + j
                    h2T_ps = pp.tile([Pn, Dn], f16, tag="psm", name="h2T_ps")
                    nc.tensor.transpose(h2T_ps[:],
                                        h2c[:, j * Pn:(j + 1) * Pn],
                                        ident16[:])
                    nc.scalar.activation(h2T_sb[:, st, :], h2T_ps[:], AF.Copy,
                                         accum_out=ln_sum[:, st:st + 1])
                    scrap = sb2.tile([Pn, Dn], f32, tag="scrap", name="scrap")
                    nc.vector.tensor_mul(scrap[:], h2T_sb[:, st, :], h2T_ps[:])
                    scr2 = sb2.tile([Pn, Dn], f32, tag="scr2", name="scr2")
                    nc.scalar.activation(scr2[:], scrap[:], AF.Copy,
                                         accum_out=ln_sq[:, st:st + 1])
                # per-tile LN stats -> per-node scale A=rstd, bias B=-mu*rstd
                mu_t = sb2.tile([Pn, 4], f32, tag="mu_t", name="mu_t")
                nc.vector.tensor_scalar(out=mu_t[:, :nbt], in0=ln_sum[:, sl],
                                        scalar1=1.0 / Dn, scalar2=None,
                                        op0=OP.mult)
                vart = sb2.tile([Pn, 4], f32, tag="vart", name="vart")
                nc.vector.tensor_scalar(out=vart[:, :nbt], in0=ln_sq[:, sl],
                                        scalar1=1.0 / Dn, scalar2=None,
                                        op0=OP.mult)
                m2v = sb2.tile([Pn, 4], f32, tag="m2v", name="m2v")
                nc.vector.tensor_mul(m2v[:, :nbt], mu_t[:, :nbt],
                                     mu_t[:, :nbt])
                nc.vector.tensor_sub(vart[:, :nbt], vart[:, :nbt],
                                     m2v[:, :nbt])
                lvt = sb2.tile([Pn, 4], f32, tag="lvt", name="lvt")
                nc.scalar.activation(lvt[:, :nbt], vart[:, :nbt], AF.Ln,
                                     bias=c_1e5[:])
                rstdt = sb2.tile([Pn, 4], f32, tag="rstdt", name="rstdt")
                nc.scalar.activation(rstdt[:, :nbt], lvt[:, :nbt], AF.Exp,
                                     scale=-0.5)
                Bt = sb2.tile([Pn, 4], f32, tag="Bt", name="Bt")
                nc.vector.tensor_scalar(out=Bt[:, :nbt], in0=mu_t[:, :nbt],
                                        scalar1=-1.0, scalar2=None,
                                        op0=OP.mult)
                nc.vector.tensor_mul(Bt[:, :nbt], Bt[:, :nbt],
                                     rstdt[:, :nbt])
                for j in range(nbt):
                    st = b0 + j
                    u = sb2.tile([Pn, Dn], f32, tag="u", name="u")
                    if p["ln_identity"]:
                        nc.scalar.activation(u[:], h2T_sb[:, st, :], AF.Relu,
                                             scale=rstdt[:, j:j + 1],
                                             bias=Bt[:, j:j + 1])
                    else:
                        nc.scalar.activation(u[:], h2T_sb[:, st, :],
                                             AF.Identity,
                                             scale=rstdt[:, j:j + 1],
                                             bias=Bt[:, j:j + 1])
                        nc.vector.tensor_mul(u[:], u[:], lng_bc[l][:])
                        nc.vector.tensor_add(u[:], u[:], lnb_bc[l][:])
                        nc.vector.tensor_scalar(out=u[:], in0=u[:],
                                                scalar1=0.0, scalar2=None,
                                                op0=OP.max)
                    nc.vector.tensor_add(h_sb[:, st, :], u[:], h_sb[:, st, :])
                if not last:
                    build_tables(b0, nbt, l + 1)
                    if b0 + nbt == HB:
                        ewA_full = table_allgather(0)
                    elif i == NMT - 1:
                        ewB_full = table_allgather(1)
                else:
                    for j in range(nbt):
                        st = b0 + j
                        hT_ps = pp.tile([Pn, Dn], f32, tag="psm",
                                        name="hT_ps")
                        nc.tensor.transpose(hT_ps[:], h_sb[:, st, :],
                                            ident[:])
                        hT = sb2.tile([Pn, Dn], f32, tag="hT", name="hT")
                        nc.scalar.activation(hT[:], hT_ps[:], AF.Copy)
                        o_ps = pp.tile([Pn, COUT], f32, tag="psm",
                                       name="o_ps")
                        nc.tensor.matmul(o_ps[:], lhsT=hT[:],
                                         rhs=out_w_sb[:],
                                         start=True, stop=True)
                        nc.scalar.activation(out_sb[:, st, :], o_ps[:],
                                             AF.Copy)
                        if not p["out_b_zero"]:
                            nc.vector.tensor_add(out_sb[:, st, :],
                                                 out_sb[:, st, :],
                                                 outb_bc[:])
            # keep pad nodes exactly zero so next-layer BN stats stay clean
            if not last:
                zero_h_pads()

        nc.sync.dma_start(
            out=out_d.rearrange("(b p) f -> p b f", p=Pn),
            in_=out_sb[:])

    _pin_act_tables()
    _fix_swdge_bump_queues(nc)
    nc.compile()
    return nc


def _fix_swdge_bump_queues(nc):
    """Tile emits the DMASW sem-bump (InstIncSwdgeSem) for prepare_only
    SWDGE preps with queue_num=0 regardless of the prep's queue. Our preps
    cycle queues exactly like Tile cycles DMASW lanes (j % 4), so lane i's
    bump belongs on queue i."""
    from concourse import bass_isa
    for b in nc.main_func.blocks:
        for i in b.instructions:
            if isinstance(i, bass_isa.InstIncSwdgeSem) and i._mode == "add":
                names = i._sem_names
                if names and names[0].startswith("DMASW"):
                    lane = int(names[0][5:].split("_")[0])
                    i.queue_num = lane % 4


def _pin_act_tables():
    """Force all activation funcs onto natural_log_exp_and_others so the
    kernel needs exactly one ACT table load (Exp/Ln/Copy/Relu/Identity are
    all members). Default placement ping-pongs exp_and_others <->
    natural_log, costing ~1.3us per switch."""
    import concourse.bacc as bacc_mod
    import concourse.hw_specs as hw_specs_mod
    if getattr(bacc_mod, "_act_tables_pinned", False):
        return
    orig = hw_specs_mod.get_activation_tables

    def pinned(arch):
        t = orig(arch)
        keep = "natural_log_exp_and_others"
        return {name: (fns if name == keep else set())
                for name, fns in t.items()}

    bacc_mod.get_activation_tables = pinned
    bacc_mod._act_tables_pinned = True


# ---------------------------------------------------------------------------
# Host-side data prep
# ---------------------------------------------------------------------------

def prep_edges(edge_index, p):
    """Group edges by (dst core, dst block), split each block's edges into an
    A segment (src table row in the first HB=24 blocks of its core's shard)
    and a B segment (remaining 25 blocks). Each half-table's global row ids
    fit int16: A has 8*3072=24576 rows, B has 8*3200=25600.

    One gather call per (block, half). Trailing pads use index 0 up to the
    cross-core max count V (so num_idxs_reg is uniform across the SPMD
    cores), then -1 (skipped by the microcode, no descriptors).

    Also builds the host-precomputed one-hot scatter matrices S:
    S[core, g, gc, slot%128, dstcol] = edge multiplicity.
    """
    Wn, Pn, NBn, GRP, HBn = p["W"], p["P"], p["NB"], p["GRP"], p["HB"]
    SHR, SHn = p["SH_REAL"], p["SH"]
    AR, BR = HBn * Pn, (NBn - HBn) * Pn
    NG = math.ceil(NBn / GRP)
    src = edge_index[0].astype(np.int64)
    dst = edge_index[1].astype(np.int64)
    score = src // SHR
    sloc = src % SHR
    core = dst // SHR
    dstl = dst % SHR
    blk = dstl // Pn
    col = (dstl % Pn).astype(np.int64)
    hi = (sloc >= AR).astype(np.int64)        # B-half flag
    # row id within its half-table
    rowid = np.where(hi == 0, score * AR + sloc, score * BR + (sloc - AR))
    # order edges by (core, block, half) then src row for DMA locality
    key = (core * NBn + blk) * 2 + hi
    order = np.lexsort((rowid, key))
    counts = np.bincount(key, minlength=Wn * NBn * 2)
    cl = counts[0::2].reshape(Wn, NBn)
    ch = counts[1::2].reshape(Wn, NBn)
    CAPA = max(1, int(math.ceil(cl.max() / Pn)))
    CAPB = max(1, int(math.ceil(ch.max() / Pn)))
    CAP = CAPA + CAPB
    starts = np.zeros(Wn * NBn * 2, np.int64)
    starts[1:] = np.cumsum(counts)[:-1]
    ne = len(src)
    ko = key[order]
    pos = np.arange(ne) - starts[ko]          # position within segment
    co = core[order]
    bo = blk[order]
    be = bo % GRP                             # block index within group
    go = bo // GRP
    ho = hi[order]
    gidx = rowid[order].astype(np.int16)
    colo = col[order]

    NBpad = NG * GRP
    # per-(core,block,half) index arrays; 0 = valid pad (gathers row 0)
    idxs_a = np.zeros((Wn, NBn, CAPA * Pn), np.int16)
    idxs_b = np.zeros((Wn, NBn, CAPB * Pn), np.int16)
    ma = ho == 0
    idxs_a[co[ma], bo[ma], pos[ma]] = gidx[ma]
    mb = ho == 1
    idxs_b[co[mb], bo[mb], pos[mb]] = gidx[mb]

    if p["exact_counts"]:
        VA = np.maximum(cl.max(axis=0), 16).astype(np.int64)   # [NB]
        VB = np.maximum(ch.max(axis=0), 16).astype(np.int64)
        for b in range(NBn):
            idxs_a[:, b, VA[b]:] = -1
            idxs_b[:, b, VB[b]:] = -1
    else:
        VA = np.full(NBn, CAPA * Pn, np.int64)
        VB = np.full(NBn, CAPB * Pn, np.int64)

    # one-hot scatter matrices: S[core, g, gc, slot%128, dstcol]
    Soh = np.zeros((Wn, NG, GRP * CAP, Pn, Pn), np.uint8)
    gc = np.where(ho == 0, be * CAPA + pos // Pn,
                  GRP * CAPA + be * CAPB + pos // Pn)
    np.add.at(Soh, (co, go, gc, pos % Pn, colo), 1)
    s_onehot = np.ascontiguousarray(
        Soh.transpose(0, 1, 3, 2, 4).reshape(Wn, NG, Pn, GRP * CAP * Pn)
    ).astype(np.float16)

    # wrapped int16 gather-index tensor, one call per (block, half):
    # [W, NG, 128, GRP*CAP*8], group row = [b0: CAPA*8|CAPB*8 | b1: ...]
    def wrap(a, capn):
        nflat = capn * Pn
        wr = np.zeros((Wn, NBn, 16, nflat // 16), np.int16)
        ii = np.arange(nflat)
        wr[:, :, ii % 16, ii // 16] = a
        return np.tile(wr, (1, 1, 8, 1))   # [W, NB, 128, capn*8]
    per_blk = np.concatenate([wrap(idxs_a, CAPA), wrap(idxs_b, CAPB)],
                             axis=3)       # [W, NB, 128, CAP*8]
    pb = np.zeros((Wn, NBpad, Pn, CAP * 8), np.int16)
    pb[:, :NBn] = per_blk
    idx16 = np.ascontiguousarray(
        pb.reshape(Wn, NG, GRP, Pn, CAP * 8).transpose(0, 1, 3, 2, 4)
        .reshape(Wn, NG, Pn, GRP * CAP * 8))
    return idx16, s_onehot, CAPA, CAPB, tuple(int(v) for v in VA), \
        tuple(int(v) for v in VB)


def prep_in_maps(inputs, p, idx16, s_onehot):
    Wn, Pn = p["W"], p["P"]
    SHR, SHn = p["SH_REAL"], p["SH"]
    x = np.asarray(inputs["x"], np.float32)
    in_maps = []
    for k in range(Wn):
        xs = np.zeros((SHn, x.shape[1]), np.float32)
        xs[:SHR] = x[k * SHR:(k + 1) * SHR]
        m = {
            "x_fm": np.ascontiguousarray(xs.T).astype(np.float16),
            "idx16": idx16[k],
            "s_onehot": s_onehot[k],
            "in_w": np.asarray(inputs["in_w"], np.float16),
            "w1": np.asarray(inputs["w1"], np.float16),
            "w2": np.asarray(inputs["w2"], np.float16),
            "bn_g": np.asarray(inputs["bn_g"], np.float32),
            "bn_b": np.asarray(inputs["bn_b"], np.float32),
            "out_w": np.asarray(inputs["out_w"], np.float32),
        }
        if not p["b2_zero"]:
            m["b2"] = np.asarray(inputs["b2"], np.float32)
        if not p["t_one"]:
            m["t"] = np.asarray(inputs["t"], np.float32)
        if not p["in_b_zero"]:
            m["in_b"] = np.asarray(inputs["in_b"], np.float32)
        if not p["out_b_zero"]:
            m["out_b"] = np.asarray(inputs["out_b"], np.float32)
        if not p["ln_identity"]:
            m["ln_g"] = np.asarray(inputs["ln_g"], np.float32)
            m["ln_b"] = np.asarray(inputs["ln_b"], np.float32)
        in_maps.append(m)
    return in_maps


def detect_fastpath(inputs, p):
    p["t_one"] = bool(np.all(np.asarray(inputs["t"]) == 1.0))
    p["in_b_zero"] = bool(np.all(np.asarray(inputs["in_b"]) == 0.0))
    p["out_b_zero"] = bool(np.all(np.asarray(inputs["out_b"]) == 0.0))
    p["b2_zero"] = bool(np.all(np.asarray(inputs["b2"]) == 0.0))
    p["ln_identity"] = bool(
        np.all(np.asarray(inputs["ln_g"]) == 1.0)
        and np.all(np.asarray(inputs["ln_b"]) == 0.0))
    # b1 is skipped unconditionally: it cancels exactly in BatchNorm.
    return p


_PROGRAM_CACHE = {}


def _get_program(p):
    key = (p["CAPL"], p["CAPH"], p["VL"], p["VH"], p["single_packet"],
           p["queue_cycle"], p["t_one"], p["in_b_zero"],
           p["out_b_zero"], p["b2_zero"], p["ln_identity"])
    if key not in _PROGRAM_CACHE:
        _PROGRAM_CACHE[key] = build_program(p)
    return _PROGRAM_CACHE[key]


def _ensure_ntff_hook():
    """Register the axon NTFF profiling hook (the image's antenv package
    lacks axon_hooks; inject an equivalent module)."""
    import types
    if "antenv.axon_hooks" in sys.modules:
        return
    sys.path.insert(0, "/root/.axon_site")
    from trn_agent_boot.trn_boot import _ntff_profile_via_ctypes
    hook = _ntff_profile_via_ctypes("/opt/axon/libaxon_pjrt.so")
    mod = types.ModuleType("antenv.axon_hooks")
    mod._hook = hook
    mod.set_axon_ntff_profile_hook = lambda h: setattr(mod, "_hook", h)
    mod.get_axon_ntff_profile_hook = lambda: mod._hook
    sys.modules["antenv.axon_hooks"] = mod


def run(inputs, trace=False, trace_cores=None):
    from concourse.bass_utils import run_bass_kernel_spmd
    if trace:
        _ensure_ntff_hook()
    p = default_params()
    detect_fastpath(inputs, p)
    idx16, s_onehot, CAPL, CAPH, VL, VH = prep_edges(
        np.asarray(inputs["edge_index"]), p)
    p["CAPL"], p["CAPH"] = CAPL, CAPH
    p["VL"], p["VH"] = VL, VH
    nc = _get_program(p)
    in_maps = prep_in_maps(inputs, p, idx16, s_onehot)
    kwargs = {}
    if trace:
        kwargs = dict(trace=True,
                      trace_cores=trace_cores or [0])
    bkr = run_bass_kernel_spmd(nc, in_maps, core_ids=list(range(p["W"])),
                               **kwargs)
    outs = []
    for k in range(p["W"]):
        outs.append(np.asarray(bkr.results[k]["out"])[:p["SH_REAL"]])
    full = np.concatenate(outs, axis=0).astype(np.float32)
    return full, bkr


def kernel(**inputs):
    full, _ = run(inputs, trace=False)
    return full
